# revision 37
# baseline (speedup 1.0000x reference)
"""MoE layer (N=4096, D=1024, H=4096, E=8, top-2) on 8 Trainium2 cores.

Strategy (expert-parallel, per the sharding hint):
  - Host computes the tiny gate (x @ Wg + bg), top-2 expert ids and softmax
    weights, then dispatches each token's row to its experts' cores
    (the host-side shard step IS the all-to-all dispatch).
  - Core e holds expert e's weights and runs the FFN for the <=C tokens
    routed to it:  y_e = relu(x_e @ W1[e] + b1[e]) @ W2[e].
  - Host combines: out[tok] += w_tok * (y_e[tok] + b2[e])  (scatter-add).

Device kernel (identical SPMD program on all 8 cores):
  - All matmul operands are bf16 (tolerance is 2e-2; output lands ~3.3e-3).
    Crucially LDWEIGHTS of a bf16 stationary tile takes ~53ns and fully
    hides under every multiply; the fp32r version lost ~15-18ns per matmul
    to exposed weight loads. Steady-state PE efficiency is ~98.5%.
  - Device capacity C=1024 = the MEAN load, so all cores do identical,
    perfectly balanced work; host absorbs routing overflow (see above).
  - gemm1: hT[h,t] += W1[dk,h].T @ xT[dk,t]  (stationary W1, moving xT),
    H in 512-wide chunks, PSUM -> relu+bias -> hts (bf16).
  - gemm2 keeps tokens as the MOVING dim: yT[d,t] += W2[hk,d].T @ hT[hk,t]
    (stationary W2). Output is yT [D, C]; the host transposes for free.
    This kills the partial-token-tile waste of a [tokens, d] layout. A
    tiny 128-token slice is ordered last to minimize the exposed tail.
  - W2 is fully SBUF-resident (8MB bf16), loaded once: no per-slab WAR.
    W1 streams in 2-deep chunk buffers; hts double-buffered across slabs.
  - All bulk DMA rides the two hardware queues (sync/scalar) with the
    startup-critical loads (first W1 chunk, x slices) FIRST in FIFO order;
    x slices get separate staging tiles so each slice's chains depend only
    on its own DMA. A short bf16 warmup chain occupies the PE while the
    startup DMAs land (the clock ramps to full ~20us into the kernel
    regardless; the warmup absorbs most of the reduced-clock era).
"""

import numpy as np
import ml_dtypes

from concourse import bacc
import concourse.mybir as mybir
from concourse.tile import TileContext
import concourse.bass_utils as bass_utils

N_TOK, D, H, E, TOPK = 4096, 1024, 4096, 8, 2
NCORES = 8
# Per-expert device token capacity = the MEAN load (8192/8). Every core then
# does identical, perfectly balanced work. The ~125 overflow pairs (1.5%) of
# the three over-subscribed experts — chosen as their smallest-gate-weight
# tokens — are computed exactly in fp32 on the host during the combine step
# (capacity-factor-1.0 dispatch with host-side overflow).
C = 1024
# token slices: (x-stage tile, offset within stage, global offset, length).
# Each slice lives in its own staging tile so a slice's chains depend only
# on that slice's DMA; slice 0 is small so the PE starts earliest.
TOK_SLICES = [(0, 0, 0, 320), (1, 0, 320, 352), (2, 0, 672, 352)]
SLAB1 = 512  # gemm1 W1 chunk width (hidden dim)
SLAB2 = 1024  # gemm2 slab width: gemm1 fills hts for a slab, gemm2 drains it
assert sum(s[3] for s in TOK_SLICES) == C

TRACE = False
TRACE_CORES = None
LAST_RESULTS = None

_NC_CACHE = {}


def _build_nc():
    f32, bf16 = mybir.dt.float32, mybir.dt.bfloat16
    nc = bacc.Bacc("TRN2", target_bir_lowering=False)
    xT = nc.dram_tensor("xT", [D, C], bf16, kind="ExternalInput")
    W1 = nc.dram_tensor("W1", [D, H], bf16, kind="ExternalInput")
    W2 = nc.dram_tensor("W2", [H, D], bf16, kind="ExternalInput")
    # b1 pre-arranged by the host as [128, H//128]: b1s[p, c] = b1[c*128+p],
    # so the load is one clean 128-byte-per-partition DMA (a [H,1] source
    # shreds into 4096 four-byte descriptors that clog a queue for ~20us).
    b1 = nc.dram_tensor("b1", [128, H // 128], f32, kind="ExternalInput")
    yT = nc.dram_tensor("yT", [D, C], f32, kind="ExternalOutput")

    n_dk = D // 128  # 8
    n_s2 = H // SLAB2  # 4 gemm2 slabs
    n_half = SLAB2 // SLAB1  # 2 gemm1 chunks per slab
    n_hm = SLAB1 // 128  # 4
    n_dm = D // 128  # 8 output-row tiles of yT
    add, mx = mybir.AluOpType.add, mybir.AluOpType.max

    W1r = W1[:, :].rearrange("(dk p) h -> p dk h", p=128)  # [128, 8, H]
    W2r = W2[:, :].rearrange("(hk p) d -> p hk d", p=128)  # [128, 32, D]
    b1r = b1[:, :]  # [128, 32]
    yTr = yT[:, :]

    with TileContext(nc) as tc:
        with (
            tc.tile_pool(name="xp", bufs=1) as xp,
            tc.tile_pool(name="w1p", bufs=2) as w1p,
            tc.tile_pool(name="w2p", bufs=1) as w2p,
            tc.tile_pool(name="hp", bufs=2) as hp,
            tc.tile_pool(name="yp", bufs=1) as yp,
            tc.tile_pool(name="cp", bufs=1) as cp,
            tc.tile_pool(name="ps1", bufs=3, space="PSUM") as ps1,
            tc.tile_pool(name="ps2", bufs=5, space="PSUM") as ps2,
        ):
            # --- PE warmup: bf16 matmuls on a zeroed tile run while the
            # startup DMAs land, so the clock gate is released (2.4 GHz)
            # when real work arrives. gpsimd memset: its queue is free
            # earliest in the preamble.
            warm = cp.tile([128, 512], bf16, name="warm")
            # split memset: the first LDWEIGHTS only needs warm[:, :128], so
            # it can start as soon as the small memset lands (subtile deps)
            nc.gpsimd.memset(warm[:, 0:128], 0.0)
            nc.gpsimd.memset(warm[:, 128:384], 0.0)
            wps = ps1.tile([128, 384], f32, tag="ps1", name="warmps")
            for i in range(12):
                nc.tensor.matmul(
                    wps[:, :384], warm[:, :128], warm[:, :384],
                    start=(i == 0), stop=(i == 11),
                )

            # --- startup DMAs: first W1 chunk + x slices, dk-halves split
            # across the two hardware queues so both pull in parallel.
            xTr = xT[:, :].rearrange("(dk p) t -> p dk t", p=128)  # [128, 8, C]
            w1t = w1p.tile([128, n_dk, SLAB1], bf16, tag="w1", name="w1t")
            nc.sync.dma_start(out=w1t[:, 0:4, :], in_=W1r[:, 0:4, 0:SLAB1])
            nc.scalar.dma_start(out=w1t[:, 4:8, :], in_=W1r[:, 4:8, 0:SLAB1])
            xstage = []
            for si, (_, _, t0, tn) in enumerate(TOK_SLICES):
                t = xp.tile([128, n_dk, tn], bf16, name=f"xs{si}")
                nc.sync.dma_start(out=t[:, 0:4, :], in_=xTr[:, 0:4, t0:t0 + tn])
                nc.scalar.dma_start(out=t[:, 4:8, :], in_=xTr[:, 4:8, t0:t0 + tn])
                xstage.append(t)
            b1s = cp.tile([128, H // 128], f32, name="b1s")
            nc.gpsimd.dma_start(out=b1s, in_=b1r)

            # W1 chunk 1: queue it right behind x so gemm1's second half
            # of slab 0 is never starved (chunks 2+ are emitted in-loop).
            w1t1 = w1p.tile([128, n_dk, SLAB1], bf16, tag="w1", name="w1t")
            nc.sync.dma_start(out=w1t1[:, :, :], in_=W1r[:, :, SLAB1:2 * SLAB1])

            # W2 fully resident: 4 slab tiles, loaded once, riding the two
            # hardware queues BEHIND the startup-critical loads (FIFO order
            # is the only reliable prioritization; gpsimd's queue races
            # ahead in program order and floods the early window).
            w2s = []
            for s2 in range(n_s2):
                t = w2p.tile([128, 8, D], bf16, tag=f"w2_{s2}", name=f"w2s{s2}")
                eng = (nc.sync, nc.scalar)[s2 % 2]
                eng.dma_start(out=t[:, 0:4, :], in_=W2r[:, s2 * 8:s2 * 8 + 4, :])
                eng.dma_start(out=t[:, 4:8, :], in_=W2r[:, s2 * 8 + 4:s2 * 8 + 8, :])
                w2s.append(t)

            yacc = [
                yp.tile([128, C], f32, tag=f"y{dm}", name=f"y{dm}")
                for dm in range(n_dm)
            ]

            _ye = [0]

            def y_eng():
                eng = (nc.sync, nc.scalar)[_ye[0] % 2]
                _ye[0] += 1
                return eng

            for s2 in range(n_s2):
                hts = [
                    hp.tile([128, C], bf16, tag=f"h{hk}", name=f"ht{hk}")
                    for hk in range(2 * n_hm)
                ]
                for half in range(n_half):
                    c = s2 * n_half + half
                    if c == 1:
                        w1t = w1t1
                    elif c > 1:  # chunks 0/1 preloaded above
                        w1t = w1p.tile(
                            [128, n_dk, SLAB1], bf16, tag="w1", name="w1t"
                        )
                        eng = (nc.sync, nc.scalar)[c % 2]
                        eng.dma_start(
                            out=w1t[:, :, :],
                            in_=W1r[:, :, c * SLAB1:(c + 1) * SLAB1],
                        )
                    for st, so, t0, tn in TOK_SLICES:
                        for hm in range(n_hm):
                            ps = ps1.tile([128, 384], f32, tag="ps1", name="ps1t")
                            for dk in range(n_dk):
                                nc.tensor.matmul(
                                    ps[:, :tn],
                                    w1t[:, dk, hm * 128:(hm + 1) * 128],
                                    xstage[st][:, dk, so:so + tn],
                                    start=(dk == 0),
                                    stop=(dk == n_dk - 1),
                                )
                            g = c * n_hm + hm  # global 128-row block of H
                            nc.vector.tensor_scalar(
                                hts[half * n_hm + hm][:, t0:t0 + tn],
                                ps[:, :tn],
                                b1s[:, g:g + 1],
                                0.0,
                                add,
                                mx,
                            )

                # gemm2: yT[dm] (+)= W2slab.T @ hts  (tokens moving).
                # Slices need not match gemm1's (subtile deps cover the
                # overlap); sizes DESCEND so the final adds + y DMAs on the
                # critical tail are the cheapest pieces.
                g2_slices = [(0, 0, 0, 512), (0, 0, 512, 384), (0, 0, 896, 128)]
                for dm in range(n_dm):
                    for _, _, t0, tn in g2_slices:
                        ps = ps2.tile([128, 512], f32, tag="ps2", name="ps2t")
                        for hk in range(2 * n_hm):
                            nc.tensor.matmul(
                                ps[:, :tn],
                                w2s[s2][:, hk, dm * 128:(dm + 1) * 128],
                                hts[hk][:, t0:t0 + tn],
                                start=(hk == 0),
                                stop=(hk == 2 * n_hm - 1),
                            )
                        ys = yacc[dm][:, t0:t0 + tn]
                        if s2 == 0:
                            nc.vector.tensor_copy(ys, ps[:, :tn])
                        else:
                            nc.vector.tensor_add(ys, ys, ps[:, :tn])
                        if s2 == n_s2 - 1 and t0 > 0:
                            # flush cols [0, t0+tn) once both leading slices'
                            # adds are done (fewer DMAs -> fewer teardown
                            # events); the tiny 128-col piece flushes last.
                            f0 = 0 if t0 == 512 else t0
                            y_eng().dma_start(
                                out=yTr[dm * 128:(dm + 1) * 128, f0:t0 + tn],
                                in_=yacc[dm][:, f0:t0 + tn],
                            )
    nc.compile()
    return nc


def _get_nc():
    if "nc" not in _NC_CACHE:
        _NC_CACHE["nc"] = _build_nc()
    return _NC_CACHE["nc"]


def kernel(x, Wg, bg, W1, b1, W2, b2):
    global LAST_RESULTS
    bf16 = ml_dtypes.bfloat16
    x = np.asarray(x, dtype=np.float32)
    Wg = np.asarray(Wg, dtype=np.float32)
    bg = np.asarray(bg, dtype=np.float32)
    W1 = np.asarray(W1, dtype=np.float32)
    b1 = np.asarray(b1, dtype=np.float32)
    W2 = np.asarray(W2, dtype=np.float32)
    b2 = np.asarray(b2, dtype=np.float32)

    # --- gate + top-k routing (replicated small gate, on host) ---
    g = x @ Wg + bg  # [N, E]
    order = np.argsort(-g, axis=1, kind="stable")[:, :TOPK]  # [N, 2]
    topv = np.take_along_axis(g, order, axis=1)
    topv = topv - topv.max(axis=1, keepdims=True)
    ex = np.exp(topv)
    sw = ex / ex.sum(axis=1, keepdims=True)  # [N, 2] softmax over selected

    nc = _get_nc()
    in_maps = []
    routing = []
    overflow = []
    for e in range(E):
        tok, kk = np.where(order == e)
        cnt = tok.size
        if cnt > C:
            # capacity-factor-1.0 dispatch: ship the C largest-weight pairs
            # to the device; the few smallest-weight overflow pairs are
            # computed exactly on the host in the combine step below.
            ordw = np.argsort(sw[tok, kk], kind="stable")
            drop, keep = ordw[:cnt - C], ordw[cnt - C:]
            overflow.append((e, tok[drop], kk[drop]))
            tok, kk = tok[keep], kk[keep]
            cnt = C
        xTe = np.zeros((D, C), bf16)
        xTe[:, :cnt] = x[tok].T.astype(bf16)
        in_maps.append(
            {
                "xT": xTe,
                "W1": np.ascontiguousarray(W1[e]).astype(bf16),
                "W2": np.ascontiguousarray(W2[e]).astype(bf16),
                "b1": np.ascontiguousarray(b1[e].reshape(H // 128, 128).T),
            }
        )
        routing.append((tok, kk, cnt))

    kwargs = {}
    if TRACE_CORES is not None:
        kwargs["trace_cores"] = TRACE_CORES
    LAST_RESULTS = bass_utils.run_bass_kernel_spmd(
        nc, in_maps, core_ids=list(range(NCORES)), trace=TRACE, **kwargs
    )

    # --- combine: scatter-add gate-weighted expert outputs ---
    out = np.zeros((N_TOK, D), np.float32)
    for e in range(E):
        tok, kk, cnt = routing[e]
        ye = LAST_RESULTS.results[e]["yT"][:, :cnt].T  # [cnt, D]
        if np.any(b2[e]):
            ye = ye + b2[e][None, :]
        # token ids are unique within one expert's list, so += is safe
        out[tok] += sw[tok, kk][:, None] * ye

    # host-side exact FFN for the few over-capacity pairs (fp32)
    for e, tok, kk in overflow:
        h = np.maximum(x[tok] @ W1[e] + b1[e], 0.0)
        ye = h @ W2[e] + b2[e]
        out[tok] += sw[tok, kk][:, None] * ye
    return out


# revision 39
# speedup vs baseline: 1.0077x; 1.0077x over previous
"""MoE layer (N=4096, D=1024, H=4096, E=8, top-2) on 8 Trainium2 cores.

Strategy (expert-parallel, per the sharding hint):
  - Host computes the tiny gate (x @ Wg + bg), top-2 expert ids and softmax
    weights, then dispatches each token's row to its experts' cores
    (the host-side shard step IS the all-to-all dispatch).
  - Core e holds expert e's weights and runs the FFN for the <=C tokens
    routed to it:  y_e = relu(x_e @ W1[e] + b1[e]) @ W2[e].
  - Host combines: out[tok] += w_tok * (y_e[tok] + b2[e])  (scatter-add).

Device kernel (identical SPMD program on all 8 cores):
  - All matmul operands are bf16 (tolerance is 2e-2; output lands ~3.3e-3).
    Crucially LDWEIGHTS of a bf16 stationary tile takes ~53ns and fully
    hides under every multiply; the fp32r version lost ~15-18ns per matmul
    to exposed weight loads. Steady-state PE efficiency is ~98.5%.
  - Device capacity C=1024 = the MEAN load, so all cores do identical,
    perfectly balanced work; host absorbs routing overflow (see above).
  - gemm1: hT[h,t] += W1[dk,h].T @ xT[dk,t]  (stationary W1, moving xT),
    H in 512-wide chunks, PSUM -> relu+bias -> hts (bf16).
  - gemm2 keeps tokens as the MOVING dim: yT[d,t] += W2[hk,d].T @ hT[hk,t]
    (stationary W2). Output is yT [D, C]; the host transposes for free.
    This kills the partial-token-tile waste of a [tokens, d] layout. A
    tiny 128-token slice is ordered last to minimize the exposed tail.
  - W2 is fully SBUF-resident (8MB bf16), loaded once: no per-slab WAR.
    W1 streams in 2-deep chunk buffers; hts double-buffered across slabs.
  - All bulk DMA rides the two hardware queues (sync/scalar) with the
    startup-critical loads (first W1 chunk, x slices) FIRST in FIFO order;
    x slices get separate staging tiles so each slice's chains depend only
    on its own DMA. A short bf16 warmup chain occupies the PE while the
    startup DMAs land (the clock ramps to full ~20us into the kernel
    regardless; the warmup absorbs most of the reduced-clock era).
"""

import numpy as np
import ml_dtypes

from concourse import bacc
import concourse.mybir as mybir
from concourse.tile import TileContext
import concourse.bass_utils as bass_utils

N_TOK, D, H, E, TOPK = 4096, 1024, 4096, 8, 2
NCORES = 8
# Per-expert device token capacity = the MEAN load (8192/8). Every core then
# does identical, perfectly balanced work. The ~125 overflow pairs (1.5%) of
# the three over-subscribed experts — chosen as their smallest-gate-weight
# tokens — are computed exactly in fp32 on the host during the combine step
# (capacity-factor-1.0 dispatch with host-side overflow).
C = 1024
# token slices: (x-stage tile, offset within stage, global offset, length).
# Each slice lives in its own staging tile so a slice's chains depend only
# on that slice's DMA; slice 0 is small so the PE starts earliest.
TOK_SLICES = [(0, 0, 0, 320), (1, 0, 320, 352), (2, 0, 672, 352)]
SLAB1 = 512  # gemm1 W1 chunk width (hidden dim)
SLAB2 = 1024  # gemm2 slab width: gemm1 fills hts for a slab, gemm2 drains it
assert sum(s[3] for s in TOK_SLICES) == C

TRACE = False
TRACE_CORES = None
LAST_RESULTS = None

_NC_CACHE = {}


def _build_nc():
    f32, bf16 = mybir.dt.float32, mybir.dt.bfloat16
    nc = bacc.Bacc("TRN2", target_bir_lowering=False)
    xT = nc.dram_tensor("xT", [D, C], bf16, kind="ExternalInput")
    W1 = nc.dram_tensor("W1", [D, H], bf16, kind="ExternalInput")
    W2 = nc.dram_tensor("W2", [H, D], bf16, kind="ExternalInput")
    # b1 pre-arranged by the host as [128, H//128]: b1s[p, c] = b1[c*128+p],
    # so the load is one clean 128-byte-per-partition DMA (a [H,1] source
    # shreds into 4096 four-byte descriptors that clog a queue for ~20us).
    b1 = nc.dram_tensor("b1", [128, H // 128], f32, kind="ExternalInput")
    yT = nc.dram_tensor("yT", [D, C], f32, kind="ExternalOutput")

    n_dk = D // 128  # 8
    n_s2 = H // SLAB2  # 4 gemm2 slabs
    n_half = SLAB2 // SLAB1  # 2 gemm1 chunks per slab
    n_hm = SLAB1 // 128  # 4
    n_dm = D // 128  # 8 output-row tiles of yT
    add, mx = mybir.AluOpType.add, mybir.AluOpType.max

    W1r = W1[:, :].rearrange("(dk p) h -> p dk h", p=128)  # [128, 8, H]
    W2r = W2[:, :].rearrange("(hk p) d -> p hk d", p=128)  # [128, 32, D]
    b1r = b1[:, :]  # [128, 32]
    yTr = yT[:, :]

    with TileContext(nc) as tc:
        with (
            tc.tile_pool(name="xp", bufs=1) as xp,
            tc.tile_pool(name="w1p", bufs=2) as w1p,
            tc.tile_pool(name="w2p", bufs=1) as w2p,
            tc.tile_pool(name="hp", bufs=2) as hp,
            tc.tile_pool(name="yp", bufs=1) as yp,
            tc.tile_pool(name="cp", bufs=1) as cp,
            tc.tile_pool(name="ps1", bufs=4, space="PSUM") as ps1,
            tc.tile_pool(name="ps2", bufs=4, space="PSUM") as ps2,
        ):
            # --- PE warmup: bf16 matmuls on a zeroed tile run while the
            # startup DMAs land, so the clock gate is released (2.4 GHz)
            # when real work arrives. gpsimd memset: its queue is free
            # earliest in the preamble.
            warm = cp.tile([128, 512], bf16, name="warm")
            # split memset: the first LDWEIGHTS only needs warm[:, :128], so
            # it can start as soon as the small memset lands (subtile deps)
            nc.gpsimd.memset(warm[:, 0:128], 0.0)
            nc.gpsimd.memset(warm[:, 128:384], 0.0)
            wps = ps1.tile([128, 384], f32, tag="ps1", name="warmps")
            for i in range(12):
                nc.tensor.matmul(
                    wps[:, :384], warm[:, :128], warm[:, :384],
                    start=(i == 0), stop=(i == 11),
                )

            # --- startup DMAs: first W1 chunk + x slices, dk-halves split
            # across the two hardware queues so both pull in parallel.
            xTr = xT[:, :].rearrange("(dk p) t -> p dk t", p=128)  # [128, 8, C]
            w1t = w1p.tile([128, n_dk, SLAB1], bf16, tag="w1", name="w1t")
            nc.sync.dma_start(out=w1t[:, 0:4, :], in_=W1r[:, 0:4, 0:SLAB1])
            nc.scalar.dma_start(out=w1t[:, 4:8, :], in_=W1r[:, 4:8, 0:SLAB1])
            xstage = []
            for si, (_, _, t0, tn) in enumerate(TOK_SLICES):
                t = xp.tile([128, n_dk, tn], bf16, name=f"xs{si}")
                nc.sync.dma_start(out=t[:, 0:4, :], in_=xTr[:, 0:4, t0:t0 + tn])
                nc.scalar.dma_start(out=t[:, 4:8, :], in_=xTr[:, 4:8, t0:t0 + tn])
                xstage.append(t)
            b1s = cp.tile([128, H // 128], f32, name="b1s")
            nc.gpsimd.dma_start(out=b1s, in_=b1r)

            # W1 chunk 1: queue it right behind x so gemm1's second half
            # of slab 0 is never starved (chunks 2+ are emitted in-loop).
            w1t1 = w1p.tile([128, n_dk, SLAB1], bf16, tag="w1", name="w1t")
            nc.sync.dma_start(out=w1t1[:, :, :], in_=W1r[:, :, SLAB1:2 * SLAB1])

            # W2 fully resident: 4 slab tiles, loaded once, riding the two
            # hardware queues BEHIND the startup-critical loads (FIFO order
            # is the only reliable prioritization; gpsimd's queue races
            # ahead in program order and floods the early window).
            w2s = []
            for s2 in range(n_s2):
                t = w2p.tile([128, 8, D], bf16, tag=f"w2_{s2}", name=f"w2s{s2}")
                eng = (nc.sync, nc.scalar)[s2 % 2]
                eng.dma_start(out=t[:, 0:4, :], in_=W2r[:, s2 * 8:s2 * 8 + 4, :])
                eng.dma_start(out=t[:, 4:8, :], in_=W2r[:, s2 * 8 + 4:s2 * 8 + 8, :])
                w2s.append(t)

            yacc = [
                yp.tile([128, C], f32, tag=f"y{dm}", name=f"y{dm}")
                for dm in range(n_dm)
            ]

            _ye = [0]

            def y_eng():
                eng = (nc.sync, nc.scalar)[_ye[0] % 2]
                _ye[0] += 1
                return eng

            for s2 in range(n_s2):
                hts = [
                    hp.tile([128, C], bf16, tag=f"h{hk}", name=f"ht{hk}")
                    for hk in range(2 * n_hm)
                ]
                for half in range(n_half):
                    c = s2 * n_half + half
                    if c == 1:
                        w1t = w1t1
                    elif c > 1:  # chunks 0/1 preloaded above
                        w1t = w1p.tile(
                            [128, n_dk, SLAB1], bf16, tag="w1", name="w1t"
                        )
                        eng = (nc.sync, nc.scalar)[c % 2]
                        eng.dma_start(
                            out=w1t[:, :, :],
                            in_=W1r[:, :, c * SLAB1:(c + 1) * SLAB1],
                        )
                    for st, so, t0, tn in TOK_SLICES:
                        for hm in range(n_hm):
                            ps = ps1.tile([128, 384], f32, tag="ps1", name="ps1t")
                            for dk in range(n_dk):
                                nc.tensor.matmul(
                                    ps[:, :tn],
                                    w1t[:, dk, hm * 128:(hm + 1) * 128],
                                    xstage[st][:, dk, so:so + tn],
                                    start=(dk == 0),
                                    stop=(dk == n_dk - 1),
                                )
                            g = c * n_hm + hm  # global 128-row block of H
                            nc.vector.tensor_scalar(
                                hts[half * n_hm + hm][:, t0:t0 + tn],
                                ps[:, :tn],
                                b1s[:, g:g + 1],
                                0.0,
                                add,
                                mx,
                            )

                # gemm2: yT[dm] (+)= W2slab.T @ hts  (tokens moving).
                # Slices need not match gemm1's (subtile deps cover the
                # overlap); sizes DESCEND so the final adds + y DMAs on the
                # critical tail are the cheapest pieces.
                g2_slices = [(0, 0, 0, 512), (0, 0, 512, 384), (0, 0, 896, 128)]
                for dm in range(n_dm):
                    for _, _, t0, tn in g2_slices:
                        ps = ps2.tile([128, 512], f32, tag="ps2", name="ps2t")
                        for hk in range(2 * n_hm):
                            nc.tensor.matmul(
                                ps[:, :tn],
                                w2s[s2][:, hk, dm * 128:(dm + 1) * 128],
                                hts[hk][:, t0:t0 + tn],
                                start=(hk == 0),
                                stop=(hk == 2 * n_hm - 1),
                            )
                        ys = yacc[dm][:, t0:t0 + tn]
                        if s2 == 0:
                            nc.vector.tensor_copy(ys, ps[:, :tn])
                        else:
                            nc.vector.tensor_add(ys, ys, ps[:, :tn])
                        if s2 == n_s2 - 1:
                            y_eng().dma_start(
                                out=yTr[dm * 128:(dm + 1) * 128, t0:t0 + tn],
                                in_=ys,
                            )
    nc.compile()
    return nc


def _get_nc():
    if "nc" not in _NC_CACHE:
        _NC_CACHE["nc"] = _build_nc()
    return _NC_CACHE["nc"]


def kernel(x, Wg, bg, W1, b1, W2, b2):
    global LAST_RESULTS
    bf16 = ml_dtypes.bfloat16
    x = np.asarray(x, dtype=np.float32)
    Wg = np.asarray(Wg, dtype=np.float32)
    bg = np.asarray(bg, dtype=np.float32)
    W1 = np.asarray(W1, dtype=np.float32)
    b1 = np.asarray(b1, dtype=np.float32)
    W2 = np.asarray(W2, dtype=np.float32)
    b2 = np.asarray(b2, dtype=np.float32)

    # --- gate + top-k routing (replicated small gate, on host) ---
    g = x @ Wg + bg  # [N, E]
    order = np.argsort(-g, axis=1, kind="stable")[:, :TOPK]  # [N, 2]
    topv = np.take_along_axis(g, order, axis=1)
    topv = topv - topv.max(axis=1, keepdims=True)
    ex = np.exp(topv)
    sw = ex / ex.sum(axis=1, keepdims=True)  # [N, 2] softmax over selected

    nc = _get_nc()
    in_maps = []
    routing = []
    overflow = []
    for e in range(E):
        tok, kk = np.where(order == e)
        cnt = tok.size
        if cnt > C:
            # capacity-factor-1.0 dispatch: ship the C largest-weight pairs
            # to the device; the few smallest-weight overflow pairs are
            # computed exactly on the host in the combine step below.
            ordw = np.argsort(sw[tok, kk], kind="stable")
            drop, keep = ordw[:cnt - C], ordw[cnt - C:]
            overflow.append((e, tok[drop], kk[drop]))
            tok, kk = tok[keep], kk[keep]
            cnt = C
        xTe = np.zeros((D, C), bf16)
        xTe[:, :cnt] = x[tok].T.astype(bf16)
        in_maps.append(
            {
                "xT": xTe,
                "W1": np.ascontiguousarray(W1[e]).astype(bf16),
                "W2": np.ascontiguousarray(W2[e]).astype(bf16),
                "b1": np.ascontiguousarray(b1[e].reshape(H // 128, 128).T),
            }
        )
        routing.append((tok, kk, cnt))

    kwargs = {}
    if TRACE_CORES is not None:
        kwargs["trace_cores"] = TRACE_CORES
    LAST_RESULTS = bass_utils.run_bass_kernel_spmd(
        nc, in_maps, core_ids=list(range(NCORES)), trace=TRACE, **kwargs
    )

    # --- combine: scatter-add gate-weighted expert outputs ---
    out = np.zeros((N_TOK, D), np.float32)
    for e in range(E):
        tok, kk, cnt = routing[e]
        ye = LAST_RESULTS.results[e]["yT"][:, :cnt].T  # [cnt, D]
        if np.any(b2[e]):
            ye = ye + b2[e][None, :]
        # token ids are unique within one expert's list, so += is safe
        out[tok] += sw[tok, kk][:, None] * ye

    # host-side exact FFN for the few over-capacity pairs (fp32)
    for e, tok, kk in overflow:
        h = np.maximum(x[tok] @ W1[e] + b1[e], 0.0)
        ye = h @ W2[e] + b2[e]
        out[tok] += sw[tok, kk][:, None] * ye
    return out
